# revision 2
# baseline (speedup 1.0000x reference)
"""LocallyConnected2d (64,64,32,32) x (1,64,64,32,32,9) -> (64,64,32,32) on 8 trn2 cores.

Strategy
--------
Spatial sharding over output rows: core i computes output rows [4i, 4i+4).

Per output location (x, y) the op is an independent GEMM:
    out[:, :, x, y] = patches(x,y) @ W(x,y).T + bias(:, x, y)
with contraction over (c, k) = 64*9 = 576, M = 64 out-channels, N = 64 batch.

On device, per location we issue 6 accumulating matmuls into PSUM:
  - x band lives in SBUF as [128, 64*204]: partitions 0-63 hold channels c
    (copy A), partitions 64-127 hold the same data shifted by +1 element
    (copy B), so a single K=128 matmul contracts over (c, two adjacent kernel
    taps) at once:
      chunk q in {0,1,2}: taps k=3q (copy A) and k=3q+1 (copy B), K=128
      single s in {0,1,2}: tap k=3s+2, K=64 (loc A on partitions 0-63,
      loc B on partitions 64-127 -- weights packed accordingly)
  - weights are host-prepacked to the exact [K, M] SBUF layout, streamed in
    8 blocks of 8 location-pairs.
  - bias is folded in with one K=8 indicator matmul per PSUM bank
    (psum[p, j*64+b] += bias_col_j[p] * ind[j, col]).

Outputs accumulate in PSUM banks of [128, 512] = 8 location-pairs, are
copied to SBUF by the vector engine and DMAed out in device-friendly
layout; the host untangles the layout at the end.

Compute dtype fp16 (fp32 accumulate in PSUM): 1 cycle/row on the PE vs 4
for fp32, and half the HBM traffic. |inputs| ~ N(0,1) so fp16 range is safe.
"""

import numpy as np

N_B, C, H, W_W, O = 64, 64, 32, 32, 64
KH = KW = 3
NCORES = 8
RPC = H // NCORES            # 4 output rows per core
BAND = RPC + 2               # 6 padded input rows per core
WP = W_W + 2                 # 34 padded width
FB = BAND * WP               # 204 free elems per batch image slice
XFREE = N_B * FB             # 13056
NPAIR_CORE = RPC * W_W // 2  # 64 location pairs per core
NTILE = 8                    # PSUM tiles per core (8 pairs each)
PAIR_COLS = 576              # weight cols per location pair
W_FREE = NPAIR_CORE * PAIR_COLS  # 36864

COMPUTE_NP = np.float16      # np.float16 | np.float32 | ml_dtypes.bfloat16

_CACHE = {}


def _mybir_dt(np_dt):
    import concourse.mybir as mybir
    import ml_dtypes

    if np_dt == np.float16:
        return mybir.dt.float16
    if np_dt == np.float32:
        return mybir.dt.float32
    if np_dt == ml_dtypes.bfloat16:
        return mybir.dt.bfloat16
    raise ValueError(np_dt)


def build_nc(compute_np=None):
    """Build the (single-program) Bass kernel; same NEFF runs on all 8 cores."""
    import concourse.bass as bass  # noqa: F401
    import concourse.mybir as mybir
    import concourse.tile as tile
    from concourse import bacc
    from contextlib import ExitStack

    cdt = _mybir_dt(compute_np or COMPUTE_NP)
    f32 = mybir.dt.float32

    nc = bacc.Bacc("TRN2", target_bir_lowering=False, debug=False)

    x_dram = nc.dram_tensor("xb", [64, XFREE], cdt, kind="ExternalInput")
    w_dram = nc.dram_tensor("wp", [128, W_FREE], cdt, kind="ExternalInput")
    b_dram = nc.dram_tensor("bp", [8, NTILE * 128], cdt, kind="ExternalInput")
    i_dram = nc.dram_tensor("ind", [8, 512], cdt, kind="ExternalInput")
    o_dram = nc.dram_tensor("out", [NTILE, 128, 512], f32, kind="ExternalOutput")

    with ExitStack() as ctx:
        tc = ctx.enter_context(tile.TileContext(nc))
        const = ctx.enter_context(tc.tile_pool(name="const", bufs=1))
        wpool = ctx.enter_context(tc.tile_pool(name="wpool", bufs=3))
        ppool = ctx.enter_context(tc.tile_pool(name="ppool", bufs=4, space="PSUM"))
        spool = ctx.enter_context(tc.tile_pool(name="spool", bufs=3))

        xsb = const.tile([128, XFREE], cdt)
        # copy A: channels on partitions 0-63
        nc.sync.dma_start(xsb[0:64, :], x_dram.ap()[:, :])
        # copy B: same data shifted by +1 element, partitions 64-127
        nc.sync.dma_start(xsb[64:128, 0 : XFREE - 1], x_dram.ap()[:, 1:])
        bias_sb = const.tile([8, NTILE * 128], cdt)
        nc.sync.dma_start(bias_sb[:], b_dram.ap()[:, :])
        ind_sb = const.tile([8, 512], cdt)
        nc.sync.dma_start(ind_sb[:], i_dram.ap()[:, :])

        x3 = xsb[:].rearrange("p (b f) -> p b f", f=FB)  # [128, 64, 204]

        for t in range(NTILE):
            wt = wpool.tile([128, 8 * PAIR_COLS], cdt)
            nc.sync.dma_start(
                wt[:], w_dram.ap()[:, t * 8 * PAIR_COLS : (t + 1) * 8 * PAIR_COLS]
            )
            ps = ppool.tile([128, 512], f32)
            xrow = t // 2
            for jp in range(8):
                jr = (t % 2) * 8 + jp       # pair index within the x-row
                phiA = xrow * WP + 2 * jr   # free offset of loc A (y = 2*jr)
                phiB = phiA + 1             # loc B (y = 2*jr + 1)
                base = jp * PAIR_COLS
                oc = jp * 64
                # loc A, paired taps (k=3q on copy A, k=3q+1 on copy B), K=128
                for q in range(3):
                    nc.tensor.matmul(
                        ps[0:64, oc : oc + 64],
                        wt[:, base + q * 64 : base + (q + 1) * 64],
                        x3[:, :, phiA + WP * q],
                        start=(jp == 0 and q == 0),
                        stop=False,
                        skip_group_check=True,
                    )
                # loc B, paired taps, K=128
                for q in range(3):
                    nc.tensor.matmul(
                        ps[64:128, oc : oc + 64],
                        wt[:, base + 192 + q * 64 : base + 192 + (q + 1) * 64],
                        x3[:, :, phiB + WP * q],
                        start=(jp == 0 and q == 0),
                        stop=False,
                        skip_group_check=True,
                    )
                # single taps k=3s+2: loc A via copy A (+2), loc B via copy B (+1)
                for s in range(3):
                    sb = base + 384 + s * 64
                    nc.tensor.matmul(
                        ps[0:64, oc : oc + 64],
                        wt[0:64, sb : sb + 64],
                        x3[0:64, :, phiA + WP * s + 2],
                        start=False,
                        stop=False,
                        skip_group_check=True,
                    )
                    nc.tensor.matmul(
                        ps[64:128, oc : oc + 64],
                        wt[64:128, sb : sb + 64],
                        x3[64:128, :, phiB + WP * s + 1],
                        start=False,
                        stop=False,
                        skip_group_check=True,
                    )
            # bias: psum[p, j*64+b] += bias[j, t*128+p] * ind[j, col]
            nc.tensor.matmul(
                ps[:, :],
                bias_sb[:, t * 128 : (t + 1) * 128],
                ind_sb[:, :],
                start=False,
                stop=True,
                skip_group_check=True,
            )
            stg = spool.tile([128, 512], f32)
            nc.vector.tensor_copy(stg[:], ps[:])
            nc.sync.dma_start(o_dram.ap()[t], stg[:])

    nc.compile()
    return nc


def pack_inputs(x, weight, bias, compute_np=None):
    """Full fp32 inputs -> list of 8 per-core input dicts (device layouts)."""
    cnp = compute_np or COMPUTE_NP
    x = np.asarray(x)
    w5 = np.asarray(weight)[0]        # (o, c, x, y, k)
    b3 = np.asarray(bias)[0]          # (o, x, y)

    xp = np.pad(x, ((0, 0), (0, 0), (1, 1), (1, 1)))  # (b, c, 34, 34)

    ind = np.zeros((8, 512), dtype=cnp)
    for j in range(8):
        ind[j, j * 64 : (j + 1) * 64] = 1.0

    in_maps = []
    for i in range(NCORES):
        band = xp[:, :, RPC * i : RPC * i + BAND, :]          # (b, c, 6, 34)
        xb = np.ascontiguousarray(band.transpose(1, 0, 2, 3)) # (c, b, 6, 34)
        xb = xb.astype(cnp).reshape(64, XFREE)

        wc = w5[:, :, RPC * i : RPC * (i + 1), :, :]          # (o, c, 4, 32, 9)
        wcr = wc.reshape(64, 64, 4, 16, 2, 9)                 # o c xh jr ab k
        chunks = wcr[..., [0, 1, 3, 4, 6, 7]].reshape(64, 64, 4, 16, 2, 3, 2)
        # -> [p=(half,c), j=(xh,jr), col=(ab,q,o)]
        chunks = chunks.transpose(6, 1, 2, 3, 4, 5, 0).reshape(128, 64, 384)
        singles = wcr[..., [2, 5, 8]]                         # o c xh jr ab s
        # -> [p=(ab,c), j=(xh,jr), col=(s,o)]
        singles = singles.transpose(4, 1, 2, 3, 5, 0).reshape(128, 64, 192)
        wp = np.concatenate([chunks, singles], axis=2)        # (128, 64, 576)
        wp = np.ascontiguousarray(wp).astype(cnp).reshape(128, W_FREE)

        bc = b3[:, RPC * i : RPC * (i + 1), :]                # (o, 4, 32)
        bcr = bc.reshape(64, 4, 2, 8, 2)                      # o xh half j' hi
        bp = bcr.transpose(3, 1, 2, 4, 0).reshape(8, NTILE * 128).astype(cnp)

        in_maps.append(
            {
                "xb": xb,
                "wp": wp,
                "bp": np.ascontiguousarray(bp),
                "ind": ind,
            }
        )
    return in_maps


def unpack_output(core_outs):
    """8 per-core [NTILE,128,512] arrays -> full (64, 64, 32, 32) output."""
    arr = np.stack(core_outs)                     # (core, t, p, col)
    arr = arr.reshape(8, 4, 2, 2, 64, 8, 64)      # core xh half hi o j' b
    out = arr.transpose(6, 4, 0, 1, 2, 5, 3)      # b o core xh half j' hi
    return np.ascontiguousarray(out.reshape(64, 64, 32, 32), dtype=np.float32)


def run_on_device(in_maps, trace=False, compute_np=None, **kwargs):
    from concourse import bass_utils

    key = ("nc", np.dtype(compute_np or COMPUTE_NP).name)
    if key not in _CACHE:
        _CACHE[key] = build_nc(compute_np)
    nc = _CACHE[key]
    res = bass_utils.run_bass_kernel_spmd(
        nc, in_maps, core_ids=list(range(NCORES)), trace=trace, **kwargs
    )
    return res


def kernel(x, weight, bias):
    in_maps = pack_inputs(x, weight, bias)
    res = run_on_device(in_maps)
    return unpack_output([r["out"] for r in res.results])


# revision 5
# speedup vs baseline: 1.0287x; 1.0287x over previous
"""LocallyConnected2d (64,64,32,32) x (1,64,64,32,32,9) -> (64,64,32,32) on 8 trn2 cores.

Strategy
--------
Spatial sharding over output rows: core i computes output rows [4i, 4i+4).

Per output location (x, y) the op is an independent GEMM:
    out[:, :, x, y] = patches(x,y) @ W(x,y).T + bias(:, x, y)
with contraction over (c, k) = 64*9 = 576, M = 64 out-channels, N = 64 batch.

On device, per location we issue 6 accumulating matmuls into PSUM:
  - x band lives in SBUF as [128, 64*204]: partitions 0-63 hold channels c
    (copy A), partitions 64-127 hold the same data shifted by +1 element
    (copy B), so a single K=128 matmul contracts over (c, two adjacent kernel
    taps) at once:
      chunk q in {0,1,2}: taps k=3q (copy A) and k=3q+1 (copy B), K=128
      single s in {0,1,2}: tap k=3s+2, K=64 (loc A on partitions 0-63,
      loc B on partitions 64-127 -- weights packed accordingly)
  - weights are host-prepacked to the exact [K, M] SBUF layout, streamed in
    8 blocks of 8 location-pairs.
  - bias is folded in with one K=8 indicator matmul per PSUM bank
    (psum[p, j*64+b] += bias_col_j[p] * ind[j, col]).

Outputs accumulate in PSUM banks of [128, 512] = 8 location-pairs, are
copied to SBUF by the vector engine and DMAed out in device-friendly
layout; the host untangles the layout at the end.

Compute dtype fp16 (fp32 accumulate in PSUM): 1 cycle/row on the PE vs 4
for fp32, and half the HBM traffic. |inputs| ~ N(0,1) so fp16 range is safe.
"""

import numpy as np

N_B, C, H, W_W, O = 64, 64, 32, 32, 64
KH = KW = 3
NCORES = 8
RPC = H // NCORES            # 4 output rows per core
BAND = RPC + 2               # 6 padded input rows per core
WP = W_W + 2                 # 34 padded width
FB = BAND * WP               # 204 free elems per batch image slice
XFREE = N_B * FB             # 13056
NPAIR_CORE = RPC * W_W // 2  # 64 location pairs per core
NTILE = 8                    # PSUM tiles per core (8 pairs each)
PAIR_COLS = 576              # weight cols per location pair
W_FREE = NPAIR_CORE * PAIR_COLS  # 36864

COMPUTE_NP = np.float16      # np.float16 | np.float32 | ml_dtypes.bfloat16

_CACHE = {}


def _mybir_dt(np_dt):
    import concourse.mybir as mybir
    import ml_dtypes

    if np_dt == np.float16:
        return mybir.dt.float16
    if np_dt == np.float32:
        return mybir.dt.float32
    if np_dt == ml_dtypes.bfloat16:
        return mybir.dt.bfloat16
    raise ValueError(np_dt)


def build_nc(compute_np=None):
    """Build the (single-program) Bass kernel; same NEFF runs on all 8 cores."""
    import concourse.bass as bass  # noqa: F401
    import concourse.mybir as mybir
    import concourse.tile as tile
    from concourse import bacc
    from contextlib import ExitStack

    cdt = _mybir_dt(compute_np or COMPUTE_NP)
    f32 = mybir.dt.float32

    nc = bacc.Bacc("TRN2", target_bir_lowering=False, debug=False)

    x_dram = nc.dram_tensor("xb", [64, XFREE], cdt, kind="ExternalInput")
    w_dram = nc.dram_tensor("wp", [128, W_FREE], cdt, kind="ExternalInput")
    b_dram = nc.dram_tensor("bp", [8, NTILE * 128], cdt, kind="ExternalInput")
    i_dram = nc.dram_tensor("ind", [8, 512], cdt, kind="ExternalInput")
    o_dram = nc.dram_tensor("out", [NTILE, 128, 512], f32, kind="ExternalOutput")

    with ExitStack() as ctx:
        tc = ctx.enter_context(tile.TileContext(nc))
        const = ctx.enter_context(tc.tile_pool(name="const", bufs=1))
        wpool = ctx.enter_context(tc.tile_pool(name="wpool", bufs=3))
        ppool = ctx.enter_context(tc.tile_pool(name="ppool", bufs=4, space="PSUM"))
        spool = ctx.enter_context(tc.tile_pool(name="spool", bufs=3))

        # x free layout is h-major: f = h*(64*34) + b*34 + w, so the first
        # half of the band (rows 0-2, needed by tiles 0-3) lands first.
        HB = 64 * WP  # 2176, one h-plane
        XH = 3 * HB   # 6528, half the band

        xsb = const.tile([128, XFREE], cdt)
        bias_sb = const.tile([8, NTILE * 128], cdt)
        ind_sb = const.tile([8, 512], cdt)
        # tiny loads off the critical HWDGE queues
        nc.gpsimd.dma_start(bias_sb[:], b_dram.ap()[:, :])
        nc.gpsimd.dma_start(ind_sb[:], i_dram.ap()[:, :])
        # copy A (channels, partitions 0-63) on SP; copy B (+1 element,
        # partitions 64-127) on ACT -- concurrent, full 16 DMA ports.
        nc.sync.dma_start(xsb[0:64, 0:XH], x_dram.ap()[:, 0:XH])
        nc.scalar.dma_start(xsb[64:128, 0:XH], x_dram.ap()[:, 1 : XH + 1])
        nc.scalar.dma_start(
            xsb[64:128, XH : XFREE - 1], x_dram.ap()[:, XH + 1 : XFREE]
        )

        x4 = xsb[:].rearrange("p (h b w) -> p h b w", h=BAND, b=64)  # [128,6,64,34]

        for t in range(NTILE):
            wt = wpool.tile([128, 8 * PAIR_COLS], cdt)
            weng = nc.sync if t % 2 == 0 else nc.scalar
            weng.dma_start(
                wt[:], w_dram.ap()[:, t * 8 * PAIR_COLS : (t + 1) * 8 * PAIR_COLS]
            )
            if t == 0:
                # second half of copy A after w0 so tile 0 can start early
                nc.sync.dma_start(xsb[0:64, XH:XFREE], x_dram.ap()[:, XH:XFREE])
            ps = ppool.tile([128, 512], f32)
            xrow = t // 2
            for jp in range(8):
                jr = (t % 2) * 8 + jp       # pair index within the x-row
                yA = 2 * jr                 # w-offset of loc A
                base = jp * PAIR_COLS
                oc = jp * 64
                # loc A, paired taps (k=3q on copy A, k=3q+1 on copy B), K=128
                for q in range(3):
                    nc.tensor.matmul(
                        ps[0:64, oc : oc + 64],
                        wt[:, base + q * 64 : base + (q + 1) * 64],
                        x4[:, xrow + q, :, yA],
                        start=(jp == 0 and q == 0),
                        stop=False,
                        skip_group_check=True,
                    )
                # loc B, paired taps, K=128
                for q in range(3):
                    nc.tensor.matmul(
                        ps[64:128, oc : oc + 64],
                        wt[:, base + 192 + q * 64 : base + 192 + (q + 1) * 64],
                        x4[:, xrow + q, :, yA + 1],
                        start=(jp == 0 and q == 0),
                        stop=False,
                        skip_group_check=True,
                    )
                # single taps k=3s+2: loc A via copy A (+2), loc B via copy B (+1)
                for s in range(3):
                    sb = base + 384 + s * 64
                    nc.tensor.matmul(
                        ps[0:64, oc : oc + 64],
                        wt[0:64, sb : sb + 64],
                        x4[0:64, xrow + s, :, yA + 2],
                        start=False,
                        stop=False,
                        skip_group_check=True,
                    )
                    nc.tensor.matmul(
                        ps[64:128, oc : oc + 64],
                        wt[64:128, sb : sb + 64],
                        x4[64:128, xrow + s, :, yA + 2],
                        start=False,
                        stop=False,
                        skip_group_check=True,
                    )
            # bias: psum[p, j*64+b] += bias[j, t*128+p] * ind[j, col]
            nc.tensor.matmul(
                ps[:, :],
                bias_sb[:, t * 128 : (t + 1) * 128],
                ind_sb[:, :],
                start=False,
                stop=True,
                skip_group_check=True,
            )
            stg = spool.tile([128, 512], f32)
            nc.vector.tensor_copy(stg[:], ps[:])
            nc.sync.dma_start(o_dram.ap()[t], stg[:])

    nc.compile()
    return nc


def pack_inputs(x, weight, bias, compute_np=None):
    """Full fp32 inputs -> list of 8 per-core input dicts (device layouts)."""
    cnp = compute_np or COMPUTE_NP
    x = np.asarray(x)
    w5 = np.asarray(weight)[0]        # (o, c, x, y, k)
    b3 = np.asarray(bias)[0]          # (o, x, y)

    xp = np.pad(x, ((0, 0), (0, 0), (1, 1), (1, 1)))  # (b, c, 34, 34)

    ind = np.zeros((8, 512), dtype=cnp)
    for j in range(8):
        ind[j, j * 64 : (j + 1) * 64] = 1.0

    in_maps = []
    for i in range(NCORES):
        band = xp[:, :, RPC * i : RPC * i + BAND, :]          # (b, c, 6, 34)
        xb = np.ascontiguousarray(band.transpose(1, 2, 0, 3)) # (c, 6, b, 34)
        xb = xb.astype(cnp).reshape(64, XFREE)

        wc = w5[:, :, RPC * i : RPC * (i + 1), :, :]          # (o, c, 4, 32, 9)
        wcr = wc.reshape(64, 64, 4, 16, 2, 9)                 # o c xh jr ab k
        chunks = wcr[..., [0, 1, 3, 4, 6, 7]].reshape(64, 64, 4, 16, 2, 3, 2)
        # -> [p=(half,c), j=(xh,jr), col=(ab,q,o)]
        chunks = chunks.transpose(6, 1, 2, 3, 4, 5, 0).reshape(128, 64, 384)
        singles = wcr[..., [2, 5, 8]]                         # o c xh jr ab s
        # -> [p=(ab,c), j=(xh,jr), col=(s,o)]
        singles = singles.transpose(4, 1, 2, 3, 5, 0).reshape(128, 64, 192)
        wp = np.concatenate([chunks, singles], axis=2)        # (128, 64, 576)
        wp = np.ascontiguousarray(wp).astype(cnp).reshape(128, W_FREE)

        bc = b3[:, RPC * i : RPC * (i + 1), :]                # (o, 4, 32)
        bcr = bc.reshape(64, 4, 2, 8, 2)                      # o xh half j' hi
        bp = bcr.transpose(3, 1, 2, 4, 0).reshape(8, NTILE * 128).astype(cnp)

        in_maps.append(
            {
                "xb": xb,
                "wp": wp,
                "bp": np.ascontiguousarray(bp),
                "ind": ind,
            }
        )
    return in_maps


def unpack_output(core_outs):
    """8 per-core [NTILE,128,512] arrays -> full (64, 64, 32, 32) output."""
    arr = np.stack(core_outs)                     # (core, t, p, col)
    arr = arr.reshape(8, 4, 2, 2, 64, 8, 64)      # core xh half hi o j' b
    out = arr.transpose(6, 4, 0, 1, 2, 5, 3)      # b o core xh half j' hi
    return np.ascontiguousarray(out.reshape(64, 64, 32, 32), dtype=np.float32)


def run_on_device(in_maps, trace=False, compute_np=None, **kwargs):
    from concourse import bass_utils

    key = ("nc", np.dtype(compute_np or COMPUTE_NP).name)
    if key not in _CACHE:
        _CACHE[key] = build_nc(compute_np)
    nc = _CACHE[key]
    res = bass_utils.run_bass_kernel_spmd(
        nc, in_maps, core_ids=list(range(NCORES)), trace=trace, **kwargs
    )
    return res


def kernel(x, weight, bias):
    in_maps = pack_inputs(x, weight, bias)
    res = run_on_device(in_maps)
    return unpack_output([r["out"] for r in res.results])
